# revision 1
# baseline (speedup 1.0000x reference)
"""Hard-negative contrastive loss on 8 TRN2 NeuronCores (Bass/Tile).

Reference semantics (B=1024, Q=32, D=512, temp scalar):
    sim[i,j,q] = fusion[i] . target[j,q];  v[i,j] = max_q sim / temp
    loss = mean_i(lse_j(v[i,:]) - v[i,i])
         + 0.5 * mean_i(log(exp(pos) + sum exp(top512 offdiag)) - pos)

Sharding: target rows j are split 128/core. Each core computes its
(1024 x 128) column block of v via fp32r matmuls (contraction d on
partitions, host-side pre-transposed inputs), AllToAll swaps blocks so
core c holds full rows c*128..c*128+127, then per-row logsumexp and
top-512 selection (threshold found by 28-step bisection on the row,
counting with a fused compare+accumulate) run locally. Host averages
the 1024 per-row losses.
"""
import sys

if "/opt/trn_rl_repo" not in sys.path:
    sys.path.insert(0, "/opt/trn_rl_repo")

import numpy as np

N_CORES = 8
B, Q, D = 1024, 32, 512
JQ = (B // N_CORES) * Q        # 4096 target vectors per core
KC = D // 128                  # 4 contraction chunks
NBLK = 512                     # jq per matmul / psum tile
JBLK = NBLK // Q               # 16 j columns per psum tile
N_ITERS = 28                   # bisection steps (range ~3 -> < fp32 ulp)
NUM_HARD = B // 2              # 512
NEG_BIG = -1.0e30

_RUNNER = None


def _build():
    import concourse.bacc as bacc
    import concourse.mybir as mybir
    import concourse.tile as tile

    f32 = mybir.dt.float32
    f32r = mybir.dt.float32r
    bf16 = mybir.dt.bfloat16
    Alu = mybir.AluOpType
    Act = mybir.ActivationFunctionType
    X = mybir.AxisListType.X

    nc = bacc.Bacc(None, target_bir_lowering=False, debug=False,
                   num_devices=N_CORES)

    fusT_ap = nc.dram_tensor("fusT", [KC, 128, B], f32, kind="ExternalInput").ap()
    tgtT_ap = nc.dram_tensor("tgtT", [KC, 128, JQ], f32, kind="ExternalInput").ap()
    oneh_ap = nc.dram_tensor("onehot", [128, B], f32, kind="ExternalInput").ap()
    out_ap = nc.dram_tensor("rowloss", [128, 2], f32, kind="ExternalOutput").ap()

    with tile.TileContext(nc) as tc:
        with (
            tc.tile_pool(name="fus", bufs=1) as fus_pool,
            tc.tile_pool(name="tgt", bufs=2) as tgt_pool,
            tc.tile_pool(name="res", bufs=1) as res_pool,
            tc.tile_pool(name="big", bufs=1) as big_pool,
            tc.tile_pool(name="small", bufs=1) as small_pool,
            tc.tile_pool(name="psum", bufs=8, space="PSUM") as psum_pool,
            tc.tile_pool(name="dram", bufs=1, space="DRAM") as dram_pool,
        ):
            # ---------- phase 1: my (1024 x 128) block of v ----------
            fus_st = fus_pool.tile([128, KC, B], f32)
            nc.sync.dma_start(fus_st[:], fusT_ap.transpose([1, 0, 2]))
            fus = fus_pool.tile([128, KC, B], f32r)
            nc.vector.tensor_copy(fus[:], fus_st[:])

            P_sb = res_pool.tile([128, N_CORES, 128], f32)  # [i_part, i_tile, j]
            for b in range(JQ // NBLK):
                tgt_st = tgt_pool.tile([128, KC, NBLK], f32)
                nc.sync.dma_start(
                    tgt_st[:],
                    tgtT_ap[:, :, b * NBLK:(b + 1) * NBLK].transpose([1, 0, 2]))
                tgt = tgt_pool.tile([128, KC, NBLK], f32r)
                nc.scalar.copy(tgt[:], tgt_st[:])
                for it in range(N_CORES):
                    ps = psum_pool.tile([128, NBLK], f32)
                    for k in range(KC):
                        nc.tensor.matmul(
                            ps[:],
                            fus[:, k, it * 128:(it + 1) * 128],
                            tgt[:, k, :],
                            start=(k == 0),
                            stop=(k == KC - 1),
                        )
                    nc.vector.reduce_max(
                        P_sb[:, it, b * JBLK:(b + 1) * JBLK],
                        ps.rearrange("p (j q) -> p j q", q=Q),
                        axis=X,
                    )

            # ---------- exchange: AllToAll column blocks -> full rows ----------
            p_in = dram_pool.tile([B, 128], f32)
            p_out = dram_pool.tile([B, 128], f32)
            for it in range(N_CORES):
                nc.sync.dma_start(p_in[it * 128:(it + 1) * 128, :], P_sb[:, it, :])
            nc.gpsimd.collective_compute(
                "AllToAll",
                Alu.bypass,
                replica_groups=[list(range(N_CORES))],
                ins=[p_in.opt()],
                outs=[p_out.opt()],
            )
            V = big_pool.tile([128, B], f32)
            for s in range(N_CORES):
                nc.sync.dma_start(V[:, s * 128:(s + 1) * 128],
                                  p_out[s * 128:(s + 1) * 128, :])

            # ---------- phase 2: per-row losses ----------
            oneh = big_pool.tile([128, B], f32)
            nc.sync.dma_start(oneh[:], oneh_ap[:])

            Vmask = big_pool.tile([128, B], f32)
            E = big_pool.tile([128, B], f32)
            junk = big_pool.tile([128, B], f32)
            junkb = big_pool.tile([128, B], bf16)

            def sm(name):
                return small_pool.tile([128, 1], f32, name=name, tag=name)

            m, negm, lo, hi, mid, cnt, cnt_hi = (
                sm(n) for n in "m negm lo hi mid cnt cnt_hi".split())
            i32 = mybir.dt.int32
            upd = small_pool.tile([128, 1], i32, name="upd", tag="upd")
            updn = small_pool.tile([128, 1], i32, name="updn", tag="updn")
            pos, sumfull, sumsel, w, epos, ew, rem, acc = (
                sm(n) for n in "pos sumfull sumsel w epos ew rem acc".split())

            nc.vector.reduce_max(m[:], V[:], axis=X)
            nc.vector.tensor_scalar_mul(negm[:], m[:], -1.0)
            nc.vector.tensor_reduce(lo[:], V[:], axis=X, op=Alu.min)
            nc.vector.tensor_scalar_add(lo[:], lo[:], -1.0)
            nc.vector.tensor_copy(hi[:], m[:])
            nc.vector.memset(cnt_hi[:], 0.0)

            # Vmask = V - 1e30 * onehot ; pos = sum(onehot * V)
            nc.vector.scalar_tensor_tensor(
                Vmask[:], oneh[:], NEG_BIG, V[:], op0=Alu.mult, op1=Alu.add)
            nc.vector.scalar_tensor_tensor(
                junk[:], oneh[:], 1.0, V[:], op0=Alu.mult, op1=Alu.mult,
                accum_out=pos[:])

            # E = exp(V - m), sumfull = sum_j E
            nc.scalar.activation(E[:], V[:], Act.Exp, bias=negm[:], scale=1.0,
                                 accum_out=sumfull[:])

            # bisection for the top-512 threshold
            for _ in range(N_ITERS):
                nc.vector.tensor_add(mid[:], lo[:], hi[:])
                nc.vector.tensor_scalar_mul(mid[:], mid[:], 0.5)
                nc.vector.tensor_scalar(
                    junkb[:], Vmask[:], mid[:], None, op0=Alu.is_gt,
                    op1=Alu.add, accum_out=cnt[:])
                nc.vector.tensor_scalar(upd[:], cnt[:], float(NUM_HARD), None,
                                        op0=Alu.is_gt)
                nc.vector.tensor_scalar(updn[:], cnt[:], float(NUM_HARD), None,
                                        op0=Alu.is_le)
                nc.vector.copy_predicated(lo[:], upd[:], mid[:])
                nc.vector.copy_predicated(hi[:], updn[:], mid[:])
                nc.vector.copy_predicated(cnt_hi[:], updn[:], cnt[:])

            # sumsel = sum E over entries with v > hi (c_hi of them)
            nc.vector.scalar_tensor_tensor(
                junk[:], Vmask[:], hi[:], E[:], op0=Alu.is_gt, op1=Alu.mult,
                accum_out=sumsel[:])
            # w = max over entries with v <= hi (exact: excluded get -200)
            nc.vector.tensor_scalar(junk[:], Vmask[:], hi[:], -200.0,
                                    op0=Alu.is_gt, op1=Alu.mult)
            nc.vector.tensor_add(junk[:], junk[:], Vmask[:])
            nc.vector.reduce_max(w[:], junk[:], axis=X)

            nc.scalar.activation(epos[:], pos[:], Act.Exp, bias=negm[:])
            nc.scalar.activation(ew[:], w[:], Act.Exp, bias=negm[:])
            # rem = 512 - cnt_hi ; acc = epos + sumsel + rem * ew
            nc.vector.tensor_scalar(rem[:], cnt_hi[:], -1.0, float(NUM_HARD),
                                    op0=Alu.mult, op1=Alu.add)
            nc.vector.tensor_mul(rem[:], rem[:], ew[:])
            nc.vector.tensor_add(acc[:], epos[:], sumsel[:])
            nc.vector.tensor_add(acc[:], acc[:], rem[:])

            outs = res_pool.tile([128, 2], f32)
            # loss_std = m + ln(sumfull) - pos ; loss_hard = m + ln(acc) - pos
            lnf, lnh, tmp = sm("lnf"), sm("lnh"), sm("tmp")
            nc.scalar.activation(lnf[:], sumfull[:], Act.Ln)
            nc.scalar.activation(lnh[:], acc[:], Act.Ln)
            nc.vector.tensor_add(tmp[:], m[:], lnf[:])
            nc.vector.tensor_sub(outs[:, 0:1], tmp[:], pos[:])
            nc.vector.tensor_add(tmp[:], m[:], lnh[:])
            nc.vector.tensor_sub(outs[:, 1:2], tmp[:], pos[:])

            nc.sync.dma_start(out_ap[:], outs[:])

    nc.compile()
    return nc


def _get_nc():
    global _RUNNER
    if _RUNNER is None:
        _RUNNER = _build()
    return _RUNNER


def make_in_maps(fusion_feats, target_feats, temp):
    fusion = np.asarray(fusion_feats, dtype=np.float32)
    target = np.asarray(target_feats, dtype=np.float32)
    scale = np.float32(1.0 / float(np.asarray(temp)))
    fusT = np.ascontiguousarray((fusion * scale).T).reshape(KC, 128, B)
    rows_per = B // N_CORES
    in_maps = []
    for c in range(N_CORES):
        shard = target[c * rows_per:(c + 1) * rows_per].reshape(JQ, D)
        tgtT = np.ascontiguousarray(shard.T).reshape(KC, 128, JQ)
        onehot = np.zeros((rows_per, B), dtype=np.float32)
        onehot[np.arange(rows_per), c * rows_per + np.arange(rows_per)] = 1.0
        in_maps.append({"fusT": fusT, "tgtT": tgtT, "onehot": onehot})
    return in_maps


def combine(results):
    rows = np.concatenate([r["rowloss"] for r in results], axis=0)  # (1024, 2)
    loss = rows[:, 0].mean(dtype=np.float32) \
        + np.float32(0.5) * rows[:, 1].mean(dtype=np.float32)
    return np.asarray(loss, dtype=np.float32)


def kernel(fusion_feats, target_feats, temp):
    from concourse import bass_utils

    nc = _get_nc()
    in_maps = make_in_maps(fusion_feats, target_feats, temp)
    res = bass_utils.run_bass_kernel_spmd(nc, in_maps, list(range(N_CORES)))
    return combine(res.results)



# revision 4
# speedup vs baseline: 25.3451x; 25.3451x over previous
"""Hard-negative contrastive loss on 8 TRN2 NeuronCores (Bass/Tile).

Reference semantics (B=1024, Q=32, D=512, temp scalar):
    sim[i,j,q] = fusion[i] . target[j,q];  v[i,j] = max_q sim / temp
    loss = mean_i(lse_j(v[i,:]) - v[i,i])
         + 0.5 * mean_i(log(exp(pos) + sum exp(top512 offdiag)) - pos)

Sharding: target rows j are split 128/core. Each core computes its
(1024 x 128) column block of v with fp8e4m3 DoubleRow matmuls
(sqrt(1/temp) folded into both operands host-side; d on partitions,
two 128-chunk pairs contracted per instruction), Q-max reduces the
psum into a bf16 block, an AllToAll swaps bf16 blocks so core c holds
full rows c*128..c*128+127, then per-row logsumexp and an approximate
top-512 threshold (10-step bisection, borderline mass folded in at
exp(hi)) run locally. Host averages the 1024 per-row losses.
fp8 + bf16 + 10 iters give rel err ~1e-4 (gate is 2e-2).
"""
import sys

if "/opt/trn_rl_repo" not in sys.path:
    sys.path.insert(0, "/opt/trn_rl_repo")

import numpy as np

N_CORES = 8
B, Q, D = 1024, 32, 512
JQ = (B // N_CORES) * Q        # 4096 target vectors per core
NBLK = 512                     # jq per matmul / psum tile
JBLK = NBLK // Q               # 16 j columns per psum tile
N_ITERS = 10                   # bisection steps
NUM_HARD = B // 2              # 512
NEG_BIG = -1.0e30

_RUNNER = None


def _build():
    import concourse.bacc as bacc
    import concourse.mybir as mybir
    import concourse.tile as tile

    f32 = mybir.dt.float32
    f8 = mybir.dt.float8e4
    bf16 = mybir.dt.bfloat16
    i32 = mybir.dt.int32
    Alu = mybir.AluOpType
    Act = mybir.ActivationFunctionType
    X = mybir.AxisListType.X
    DR = mybir.MatmulPerfMode.DoubleRow

    nc = bacc.Bacc(None, target_bir_lowering=False, debug=False,
                   num_devices=N_CORES)

    fus_ap = nc.dram_tensor("fus8", [2, 2, 128, B], f8, kind="ExternalInput").ap()
    tgt_ap = nc.dram_tensor("tgt8", [2, 2, 128, JQ], f8, kind="ExternalInput").ap()
    oneh_ap = nc.dram_tensor("onehot", [128, B], bf16, kind="ExternalInput").ap()
    out_ap = nc.dram_tensor("rowloss", [128, 2], f32, kind="ExternalOutput").ap()

    with tile.TileContext(nc) as tc:
        with (
            tc.tile_pool(name="fus", bufs=1) as fus_pool,
            tc.tile_pool(name="tgt", bufs=1) as tgt_pool,
            tc.tile_pool(name="res", bufs=1) as res_pool,
            tc.tile_pool(name="big", bufs=1) as big_pool,
            tc.tile_pool(name="small", bufs=1) as small_pool,
            tc.tile_pool(name="psum", bufs=8, space="PSUM") as psum_pool,
            tc.tile_pool(name="dram", bufs=1, space="DRAM") as dram_pool,
        ):
            # ---------- phase 1: my (1024 x 128) block of v ----------
            fus = fus_pool.tile([128, 2, 2, B], f8)
            nc.sync.dma_start(fus[:], fus_ap.transpose([2, 0, 1, 3]))
            tgt = tgt_pool.tile([128, 2, 2, JQ], f8)
            nc.sync.dma_start(tgt[:], tgt_ap.transpose([2, 0, 1, 3]))

            P = res_pool.tile([128, N_CORES, 128], bf16)  # [i_part, i_tile, j]
            for it in range(N_CORES):
                for b in range(JQ // NBLK):
                    ps = psum_pool.tile([128, NBLK], f32)
                    for kp in range(2):
                        nc.tensor.matmul(
                            ps[:],
                            fus[:, kp, :, it * 128:(it + 1) * 128],
                            tgt[:, kp, :, b * NBLK:(b + 1) * NBLK],
                            start=(kp == 0),
                            stop=(kp == 1),
                            perf_mode=DR,
                        )
                    nc.vector.reduce_max(
                        P[:, it, b * JBLK:(b + 1) * JBLK],
                        ps.rearrange("p (j q) -> p j q", q=Q),
                        axis=X,
                    )

            # ---------- exchange: AllToAll column blocks -> full rows ----------
            p_in = dram_pool.tile([B, 128], bf16)
            p_out = dram_pool.tile([B, 128], bf16)
            for it in range(N_CORES):
                nc.sync.dma_start(p_in[it * 128:(it + 1) * 128, :], P[:, it, :])
            nc.gpsimd.collective_compute(
                "AllToAll",
                Alu.bypass,
                replica_groups=[list(range(N_CORES))],
                ins=[p_in.opt()],
                outs=[p_out.opt()],
            )
            V = big_pool.tile([128, B], bf16)
            for s in range(N_CORES):
                nc.sync.dma_start(V[:, s * 128:(s + 1) * 128],
                                  p_out[s * 128:(s + 1) * 128, :])

            # ---------- phase 2: per-row losses ----------
            oneh = big_pool.tile([128, B], bf16)
            nc.sync.dma_start(oneh[:], oneh_ap[:])

            Vmask = big_pool.tile([128, B], bf16)
            E = big_pool.tile([128, B], bf16)
            junk = big_pool.tile([128, B], bf16)
            junkb = big_pool.tile([128, B], bf16)

            def sm(name, dt=f32):
                return small_pool.tile([128, 1], dt, name=name, tag=name)

            m, negm, lo, hi, mid, cnt, cnt_hi = (
                sm(n) for n in "m negm lo hi mid cnt cnt_hi".split())
            upd = sm("upd", i32)
            updn = sm("updn", i32)
            pos, sumfull, sumsel, epos, ehi, rem, acc = (
                sm(n) for n in "pos sumfull sumsel epos ehi rem acc".split())

            nc.vector.reduce_max(m[:], V[:], axis=X)
            nc.vector.tensor_scalar_mul(negm[:], m[:], -1.0)
            nc.vector.tensor_reduce(lo[:], V[:], axis=X, op=Alu.min)
            nc.vector.tensor_scalar_add(lo[:], lo[:], -1.0)
            nc.vector.tensor_copy(hi[:], m[:])

            # Vmask = V - 1e30 * onehot ; pos = sum(onehot * V)
            nc.vector.scalar_tensor_tensor(
                Vmask[:], oneh[:], NEG_BIG, V[:], op0=Alu.mult, op1=Alu.add)
            nc.vector.scalar_tensor_tensor(
                junk[:], oneh[:], 1.0, V[:], op0=Alu.mult, op1=Alu.mult,
                accum_out=pos[:])

            # E = exp(V - m), sumfull = sum_j E
            nc.scalar.activation(E[:], V[:], Act.Exp, bias=negm[:], scale=1.0,
                                 accum_out=sumfull[:])

            # bisection for the top-512 threshold
            for _ in range(N_ITERS):
                nc.vector.tensor_add(mid[:], lo[:], hi[:])
                nc.vector.tensor_scalar_mul(mid[:], mid[:], 0.5)
                nc.vector.tensor_scalar(
                    junkb[:], Vmask[:], mid[:], None, op0=Alu.is_gt,
                    op1=Alu.add, accum_out=cnt[:])
                nc.vector.tensor_scalar(upd[:], cnt[:], float(NUM_HARD), None,
                                        op0=Alu.is_gt)
                nc.vector.tensor_scalar(updn[:], cnt[:], float(NUM_HARD), None,
                                        op0=Alu.is_le)
                nc.vector.copy_predicated(lo[:], upd[:], mid[:])
                nc.vector.copy_predicated(hi[:], updn[:], mid[:])

            # cnt_hi = #{v > hi};  sumsel = sum E over those entries
            nc.vector.tensor_scalar(
                junkb[:], Vmask[:], hi[:], None, op0=Alu.is_gt,
                op1=Alu.add, accum_out=cnt_hi[:])
            nc.vector.scalar_tensor_tensor(
                junk[:], Vmask[:], hi[:], E[:], op0=Alu.is_gt, op1=Alu.mult,
                accum_out=sumsel[:])

            # acc = epos + sumsel + (512 - cnt_hi) * exp(hi - m)
            nc.scalar.activation(epos[:], pos[:], Act.Exp, bias=negm[:])
            nc.scalar.activation(ehi[:], hi[:], Act.Exp, bias=negm[:])
            nc.vector.tensor_scalar(rem[:], cnt_hi[:], -1.0, float(NUM_HARD),
                                    op0=Alu.mult, op1=Alu.add)
            nc.vector.tensor_mul(rem[:], rem[:], ehi[:])
            nc.vector.tensor_add(acc[:], epos[:], sumsel[:])
            nc.vector.tensor_add(acc[:], acc[:], rem[:])

            outs = res_pool.tile([128, 2], f32)
            # loss_std = m + ln(sumfull) - pos ; loss_hard = m + ln(acc) - pos
            lnf, lnh, tmp = sm("lnf"), sm("lnh"), sm("tmp")
            nc.scalar.activation(lnf[:], sumfull[:], Act.Ln)
            nc.scalar.activation(lnh[:], acc[:], Act.Ln)
            nc.vector.tensor_add(tmp[:], m[:], lnf[:])
            nc.vector.tensor_sub(outs[:, 0:1], tmp[:], pos[:])
            nc.vector.tensor_add(tmp[:], m[:], lnh[:])
            nc.vector.tensor_sub(outs[:, 1:2], tmp[:], pos[:])

            nc.sync.dma_start(out_ap[:], outs[:])

    nc.compile()
    return nc


def _get_nc():
    global _RUNNER
    if _RUNNER is None:
        _RUNNER = _build()
    return _RUNNER


def make_in_maps(fusion_feats, target_feats, temp):
    import ml_dtypes

    f8 = ml_dtypes.float8_e4m3
    fusion = np.asarray(fusion_feats, dtype=np.float32)
    target = np.asarray(target_feats, dtype=np.float32)
    scale = np.float32(1.0 / np.sqrt(float(np.asarray(temp))))
    # d -> (kp, pair, p): d = kp*256 + pair*128 + p
    fus8 = np.ascontiguousarray((fusion * scale).T).reshape(2, 2, 128, B)
    fus8 = fus8.astype(f8)
    rows_per = B // N_CORES
    in_maps = []
    for c in range(N_CORES):
        shard = target[c * rows_per:(c + 1) * rows_per].reshape(JQ, D)
        tgt8 = np.ascontiguousarray((shard * scale).T).reshape(2, 2, 128, JQ)
        tgt8 = tgt8.astype(f8)
        onehot = np.zeros((rows_per, B), dtype=ml_dtypes.bfloat16)
        onehot[np.arange(rows_per), c * rows_per + np.arange(rows_per)] = 1.0
        in_maps.append({"fus8": fus8, "tgt8": tgt8, "onehot": onehot})
    return in_maps


def combine(results):
    rows = np.concatenate([r["rowloss"] for r in results], axis=0)  # (1024, 2)
    loss = rows[:, 0].mean(dtype=np.float32) \
        + np.float32(0.5) * rows[:, 1].mean(dtype=np.float32)
    return np.asarray(loss, dtype=np.float32)


def kernel(fusion_feats, target_feats, temp):
    from concourse import bass_utils

    nc = _get_nc()
    in_maps = make_in_maps(fusion_feats, target_feats, temp)
    res = bass_utils.run_bass_kernel_spmd(nc, in_maps, list(range(N_CORES)))
    return combine(res.results)


# revision 5
# speedup vs baseline: 30.9967x; 1.2230x over previous
"""Hard-negative contrastive loss on 8 TRN2 NeuronCores (Bass/Tile).

Reference semantics (B=1024, Q=32, D=512, temp scalar):
    sim[i,j,q] = fusion[i] . target[j,q];  v[i,j] = max_q sim / temp
    loss = mean_i(lse_j(v[i,:]) - v[i,i])
         + 0.5 * mean_i(log(exp(pos) + sum exp(top512 offdiag)) - pos)

Sharding: target rows j are split 128/core. Each core computes its
(1024 x 128) column block of v with fp8e4m3 DoubleRow matmuls
(sqrt(1/temp) folded into both operands host-side; d on partitions,
two 128-chunk pairs contracted per instruction; operands arrive in
partition-major layout so DMAs are contiguous). The Q-max runs on DVE
into fp32, the scalar engine casts each i-tile to bf16 and stages it
for an AllToAll that gives core c its full rows c*128..+127. Per-row
logsumexp and an approximate top-512 threshold (6-step bisection;
borderline mass folded in at exp(hi)) reduce each row to 6 stats;
the host finishes the per-row losses and averages. fp8 + bf16 + 6
iters give rel err ~1e-4 (gate is 2e-2).
"""
import sys

if "/opt/trn_rl_repo" not in sys.path:
    sys.path.insert(0, "/opt/trn_rl_repo")

import numpy as np

N_CORES = 8
B, Q, D = 1024, 32, 512
JQ = (B // N_CORES) * Q        # 4096 target vectors per core
NBLK = 512                     # jq per matmul / psum tile
JBLK = NBLK // Q               # 16 j columns per psum tile
N_ITERS = 6                    # bisection steps
NUM_HARD = B // 2              # 512
NEG_BIG = -1.0e30

_RUNNER = None


def _build():
    import concourse.bacc as bacc
    import concourse.mybir as mybir
    import concourse.tile as tile

    f32 = mybir.dt.float32
    f8 = mybir.dt.float8e4
    bf16 = mybir.dt.bfloat16
    i32 = mybir.dt.int32
    Alu = mybir.AluOpType
    Act = mybir.ActivationFunctionType
    X = mybir.AxisListType.X
    DR = mybir.MatmulPerfMode.DoubleRow

    nc = bacc.Bacc(None, target_bir_lowering=False, debug=False,
                   num_devices=N_CORES)

    fus_ap = nc.dram_tensor("fus8", [128, 2, 2, B], f8, kind="ExternalInput").ap()
    tgt_ap = nc.dram_tensor("tgt8", [128, 2, 2, JQ], f8, kind="ExternalInput").ap()
    oneh_ap = nc.dram_tensor("onehot", [128, B], bf16, kind="ExternalInput").ap()
    out_ap = nc.dram_tensor("rowstats", [128, 6], f32, kind="ExternalOutput").ap()

    with tile.TileContext(nc) as tc:
        with (
            tc.tile_pool(name="fus", bufs=1) as fus_pool,
            tc.tile_pool(name="tgt", bufs=1) as tgt_pool,
            tc.tile_pool(name="res", bufs=1) as res_pool,
            tc.tile_pool(name="big", bufs=1) as big_pool,
            tc.tile_pool(name="small", bufs=1) as small_pool,
            tc.tile_pool(name="psum", bufs=8, space="PSUM") as psum_pool,
            tc.tile_pool(name="dram", bufs=1, space="DRAM") as dram_pool,
        ):
            # ---------- phase 1: my (1024 x 128) block of v ----------
            fus = fus_pool.tile([128, 2, 2, B], f8)
            nc.sync.dma_start(fus[:], fus_ap[:])
            tgt = tgt_pool.tile([128, 2, 2, JQ], f8)
            for kp in range(2):
                nc.sync.dma_start(tgt[:, kp], tgt_ap[:, kp])

            P32 = res_pool.tile([128, N_CORES, 128], f32)   # [i_part, i_tile, j]
            Pb = res_pool.tile([128, N_CORES, 128], bf16)
            p_in = dram_pool.tile([B, 128], bf16)
            p_out = dram_pool.tile([B, 128], bf16)

            for it in range(N_CORES):
                for b in range(JQ // NBLK):
                    ps = psum_pool.tile([128, NBLK], f32)
                    for kp in range(2):
                        nc.tensor.matmul(
                            ps[:],
                            fus[:, kp, :, it * 128:(it + 1) * 128],
                            tgt[:, kp, :, b * NBLK:(b + 1) * NBLK],
                            start=(kp == 0),
                            stop=(kp == 1),
                            perf_mode=DR,
                        )
                    nc.vector.reduce_max(
                        P32[:, it, b * JBLK:(b + 1) * JBLK],
                        ps.rearrange("p (j q) -> p j q", q=Q),
                        axis=X,
                    )
                # cast this i-tile to bf16 (scalar engine) and stage it for
                # the AllToAll so exchange DMAs overlap the remaining tiles
                nc.scalar.copy(Pb[:, it], P32[:, it])
                nc.sync.dma_start(p_in[it * 128:(it + 1) * 128, :], Pb[:, it])

            # ---------- exchange: AllToAll column blocks -> full rows ----------
            nc.gpsimd.collective_compute(
                "AllToAll",
                Alu.bypass,
                replica_groups=[list(range(N_CORES))],
                ins=[p_in.opt()],
                outs=[p_out.opt()],
            )
            V = big_pool.tile([128, B], bf16)
            for s in range(N_CORES):
                nc.sync.dma_start(V[:, s * 128:(s + 1) * 128],
                                  p_out[s * 128:(s + 1) * 128, :])

            # ---------- phase 2: per-row stats ----------
            oneh = big_pool.tile([128, B], bf16)
            nc.sync.dma_start(oneh[:], oneh_ap[:])

            Vmask = big_pool.tile([128, B], f32)
            E = big_pool.tile([128, B], bf16)
            junk = big_pool.tile([128, B], bf16)
            junkf = big_pool.tile([128, B], f32)

            outs = res_pool.tile([128, 6], f32)  # m pos sumfull sumsel cnt_hi hi
            m = outs[:, 0:1]
            pos = outs[:, 1:2]
            sumfull = outs[:, 2:3]
            sumsel = outs[:, 3:4]
            cnt_hi = outs[:, 4:5]

            def sm(name, dt=f32):
                return small_pool.tile([128, 1], dt, name=name, tag=name)

            negm, lo, hi, mid, cnt = (sm(n) for n in "negm lo hi mid cnt".split())
            upd = sm("upd", i32)
            updn = sm("updn", i32)

            nc.vector.reduce_max(m, V[:], axis=X)
            nc.vector.tensor_scalar_mul(negm[:], m, -1.0)
            nc.vector.tensor_reduce(lo[:], V[:], axis=X, op=Alu.min)
            nc.vector.tensor_scalar_add(lo[:], lo[:], -1.0)
            nc.vector.tensor_copy(hi[:], m)

            # Vmask = V - 1e30 * onehot ; pos = sum(onehot * V)
            nc.vector.scalar_tensor_tensor(
                Vmask[:], oneh[:], NEG_BIG, V[:], op0=Alu.mult, op1=Alu.add)
            nc.vector.scalar_tensor_tensor(
                junk[:], oneh[:], 1.0, V[:], op0=Alu.mult, op1=Alu.mult,
                accum_out=pos)

            # E = exp(V - m), sumfull = sum_j E
            nc.scalar.activation(E[:], V[:], Act.Exp, bias=negm[:], scale=1.0,
                                 accum_out=sumfull)

            # bisection for the top-512 threshold
            for _ in range(N_ITERS):
                nc.vector.tensor_add(mid[:], lo[:], hi[:])
                nc.vector.tensor_scalar_mul(mid[:], mid[:], 0.5)
                nc.vector.tensor_scalar(
                    junkf[:], Vmask[:], mid[:], None, op0=Alu.is_gt,
                    op1=Alu.add, accum_out=cnt[:])
                nc.vector.tensor_scalar(upd[:], cnt[:], float(NUM_HARD), None,
                                        op0=Alu.is_gt)
                nc.vector.tensor_scalar(updn[:], cnt[:], float(NUM_HARD), None,
                                        op0=Alu.is_le)
                nc.vector.copy_predicated(lo[:], upd[:], mid[:])
                nc.vector.copy_predicated(hi[:], updn[:], mid[:])

            # cnt_hi = #{v > hi};  sumsel = sum E over those entries
            nc.vector.tensor_scalar(
                junkf[:], Vmask[:], hi[:], None, op0=Alu.is_gt,
                op1=Alu.add, accum_out=cnt_hi)
            nc.vector.scalar_tensor_tensor(
                junkf[:], Vmask[:], hi[:], E[:], op0=Alu.is_gt, op1=Alu.mult,
                accum_out=sumsel)
            nc.vector.tensor_copy(outs[:, 5:6], hi[:])

            nc.sync.dma_start(out_ap[:], outs[:])

    nc.compile()
    return nc


def _get_nc():
    global _RUNNER
    if _RUNNER is None:
        _RUNNER = _build()
    return _RUNNER


def make_in_maps(fusion_feats, target_feats, temp):
    import ml_dtypes

    f8 = ml_dtypes.float8_e4m3
    fusion = np.asarray(fusion_feats, dtype=np.float32)
    target = np.asarray(target_feats, dtype=np.float32)
    scale = np.float32(1.0 / np.sqrt(float(np.asarray(temp))))
    # d -> (kp, pair, p): d = kp*256 + pair*128 + p; partition-major layout
    fus8 = np.ascontiguousarray(
        (fusion * scale).T.reshape(2, 2, 128, B).transpose(2, 0, 1, 3)
    ).astype(f8)
    rows_per = B // N_CORES
    in_maps = []
    for c in range(N_CORES):
        shard = target[c * rows_per:(c + 1) * rows_per].reshape(JQ, D)
        tgt8 = np.ascontiguousarray(
            (shard * scale).T.reshape(2, 2, 128, JQ).transpose(2, 0, 1, 3)
        ).astype(f8)
        onehot = np.zeros((rows_per, B), dtype=ml_dtypes.bfloat16)
        onehot[np.arange(rows_per), c * rows_per + np.arange(rows_per)] = 1.0
        in_maps.append({"fus8": fus8, "tgt8": tgt8, "onehot": onehot})
    return in_maps


def combine(results):
    rows = np.concatenate([r["rowstats"] for r in results], axis=0)  # (1024, 6)
    m, pos, sumfull, sumsel, cnt_hi, hi = (rows[:, k].astype(np.float64)
                                           for k in range(6))
    epos = np.exp(pos - m)
    ehi = np.exp(hi - m)
    acc = epos + sumsel + (NUM_HARD - cnt_hi) * ehi
    loss_std = (m + np.log(sumfull) - pos).mean()
    loss_hard = (m + np.log(acc) - pos).mean()
    return np.asarray(loss_std + 0.5 * loss_hard, dtype=np.float32)


def kernel(fusion_feats, target_feats, temp):
    from concourse import bass_utils

    nc = _get_nc()
    in_maps = make_in_maps(fusion_feats, target_feats, temp)
    res = bass_utils.run_bass_kernel_spmd(nc, in_maps, list(range(N_CORES)))
    return combine(res.results)
